# revision 1
# baseline (speedup 1.0000x reference)
"""Trainium2 Bass kernel for nn_LogitLayer: out = exp(-r * (N[i] - N[j] + v)).

Self-contained: accepts FULL inputs, shards nonzeros across 8 NeuronCores
data-parallel, replicates the node_constants table on every core, runs a
Bass/Tile kernel per core, and reassembles the full output.

Device algorithm (per core, per tile of 32768 nonzeros):
  - node_constants is stored 16-way interleaved across each group of 16 SBUF
    partitions: tab[16g+p, n] = N[16n + p] (identical for all 8 groups).
  - For each index stream (i and j): GPSIMD ap_gather with q = idx >> 4
    fetches candidate columns C[16g+p, k] = N[16*q_{g,k} + p].
  - A DVE mask (p % 16 == idx & 15) one-hots the wanted partition, and a
    static block-diagonal +/-1 PE matmul reduces each 16-partition group,
    computing sel_i - sel_j directly in PSUM.
  - DVE adds v, ACT applies exp with scale = -rationality, DMA out.

Host-side work is layout-only: int64->int32 views, bit-split of indices into
(q, r), wrap/replicate reshapes, zero-padding. All value math (table reads,
subtract, add, exp) runs on device.
"""

import os

import numpy as np

NNZ = 20_000_000
NNODES = 100_000
NCORES = 8
K = 4096  # ap_gather indices per 16-partition group per instruction
ET = 8 * K  # nonzeros per tile
NNZ_CORE = NNZ // NCORES
T = (NNZ_CORE + ET - 1) // ET  # tiles per core
PAD = T * ET
NE = NNODES // 16  # table entries per partition (16-way interleave)
P = 128

# Set to "1" by test harnesses to capture an NTFF profile; leaves a module
# global with the measured executed-kernel time.
LAST_EXEC_NS = None


def _install_ntff_hook():
    import sys
    import types

    if "antenv.axon_hooks" in sys.modules:
        return
    mod = types.ModuleType("antenv.axon_hooks")
    state = {"hook": None}
    mod.set_axon_ntff_profile_hook = lambda h: state.__setitem__("hook", h)
    mod.get_axon_ntff_profile_hook = lambda: state["hook"]
    sys.modules["antenv.axon_hooks"] = mod
    try:
        from trn_agent_boot.trn_boot import _ntff_profile_via_ctypes

        mod.set_axon_ntff_profile_hook(
            _ntff_profile_via_ctypes("/opt/axon/libaxon_pjrt.so")
        )
    except Exception:
        pass


def _build():
    import concourse.bacc as bacc
    import concourse.mybir as mybir
    from concourse.tile import TileContext

    f32 = mybir.dt.float32
    nc = bacc.Bacc("TRN2")
    tab = nc.dram_tensor("tab", [P, NE], f32, kind="ExternalInput")
    qwi = nc.dram_tensor("qwi", [T, P, K // 16], mybir.dt.int16, kind="ExternalInput")
    qwj = nc.dram_tensor("qwj", [T, P, K // 16], mybir.dt.int16, kind="ExternalInput")
    rri = nc.dram_tensor("rri", [T, P, K], mybir.dt.uint8, kind="ExternalInput")
    rrj = nc.dram_tensor("rrj", [T, P, K], mybir.dt.uint8, kind="ExternalInput")
    vals = nc.dram_tensor("vals", [T, 8, K], f32, kind="ExternalInput")
    iota = nc.dram_tensor("iota", [P, 1], f32, kind="ExternalInput")
    redw = nc.dram_tensor("redw", [P, 16], f32, kind="ExternalInput")
    negr = nc.dram_tensor("negr", [P, 1], f32, kind="ExternalInput")
    y = nc.dram_tensor("y", [T, 8, K], f32, kind="ExternalOutput")

    with TileContext(nc) as tc:
        with (
            tc.tile_pool(name="const", bufs=1) as cpool,
            tc.tile_pool(name="qio", bufs=3) as qio,
            tc.tile_pool(name="rio", bufs=2) as rio,
            tc.tile_pool(name="vio", bufs=2) as vio,
            tc.tile_pool(name="cand", bufs=3) as cand,
            tc.tile_pool(name="mwork", bufs=2) as mwork,
            tc.tile_pool(name="owork", bufs=2) as owork,
            tc.tile_pool(name="psum", bufs=1, space="PSUM") as pp,
        ):
            tab_t = cpool.tile([P, NE], f32)
            nc.sync.dma_start(out=tab_t[:], in_=tab[:])
            iota_t = cpool.tile([P, 1], f32)
            nc.sync.dma_start(out=iota_t[:], in_=iota[:])
            redw_t = cpool.tile([P, 16], f32)
            nc.sync.dma_start(out=redw_t[:], in_=redw[:])
            negr_t = cpool.tile([P, 1], f32)
            nc.sync.dma_start(out=negr_t[:], in_=negr[:])
            tab3 = tab_t[:].rearrange("p (n d) -> p n d", d=1)

            for t in range(T):
                qwi_t = qio.tile([P, K // 16], mybir.dt.int16, tag="qwi")
                nc.sync.dma_start(out=qwi_t[:], in_=qwi[:][t])
                qwj_t = qio.tile([P, K // 16], mybir.dt.int16, tag="qwj")
                nc.sync.dma_start(out=qwj_t[:], in_=qwj[:][t])
                rri_t = rio.tile([P, K], mybir.dt.uint8, tag="rri")
                nc.sync.dma_start(out=rri_t[:], in_=rri[:][t])
                rrj_t = rio.tile([P, K], mybir.dt.uint8, tag="rrj")
                nc.sync.dma_start(out=rrj_t[:], in_=rrj[:][t])

                ci = cand.tile([P, K], f32, tag="c")
                nc.gpsimd.ap_gather(
                    out_ap=ci[:], in_ap=tab3, idxs_ap=qwi_t[:],
                    channels=P, num_elems=NE, d=1, num_idxs=K,
                )
                cj = cand.tile([P, K], f32, tag="c")
                nc.gpsimd.ap_gather(
                    out_ap=cj[:], in_ap=tab3, idxs_ap=qwj_t[:],
                    channels=P, num_elems=NE, d=1, num_idxs=K,
                )

                mi = mwork.tile([P, K], f32, tag="m")
                nc.vector.tensor_scalar(
                    out=mi[:], in0=rri_t[:], scalar1=iota_t[:, 0:1], scalar2=None,
                    op0=mybir.AluOpType.is_equal,
                )
                nc.vector.tensor_tensor(
                    out=ci[:], in0=ci[:], in1=mi[:], op=mybir.AluOpType.mult
                )
                mj = mwork.tile([P, K], f32, tag="m")
                nc.vector.tensor_scalar(
                    out=mj[:], in0=rrj_t[:], scalar1=iota_t[:, 0:1], scalar2=None,
                    op0=mybir.AluOpType.is_equal,
                )
                nc.vector.tensor_tensor(
                    out=cj[:], in0=cj[:], in1=mj[:], op=mybir.AluOpType.mult
                )

                ps = pp.tile([8, K], f32, tag="ps")
                for c in range(K // 512):
                    sl = slice(c * 512, (c + 1) * 512)
                    nc.tensor.matmul(
                        out=ps[:, sl], lhsT=redw_t[:, 0:8], rhs=ci[:, sl],
                        start=True, stop=False,
                    )
                    nc.tensor.matmul(
                        out=ps[:, sl], lhsT=redw_t[:, 8:16], rhs=cj[:, sl],
                        start=False, stop=True,
                    )

                CH = K // 4
                for c in range(4):
                    sl = slice(c * CH, (c + 1) * CH)
                    v_t = vio.tile([8, CH], f32, tag="v")
                    nc.sync.dma_start(out=v_t[:], in_=vals[:][t][:, sl])
                    s_t = owork.tile([8, CH], f32, tag="s")
                    nc.vector.tensor_tensor(
                        out=s_t[:], in0=ps[:, sl], in1=v_t[:], op=mybir.AluOpType.add
                    )
                    nc.scalar.activation(
                        s_t[:], s_t[:], mybir.ActivationFunctionType.Exp,
                        scale=negr_t[0:8, 0:1],
                    )
                    nc.sync.dma_start(out=y[:][t][:, sl], in_=s_t[:])
    nc.finalize()
    return nc


def _prep_stream(idx_i32):
    """idx (PAD,) int32 -> (q wrapped [T,128,K//16] i16, r replicated [T,128,K] u8)."""
    q16 = (idx_i32 >> 4).astype(np.int16)
    r8 = (idx_i32 & 15).astype(np.uint8)
    qw = q16.reshape(T, 8, K // 16, 16).transpose(0, 1, 3, 2).reshape(T, P, K // 16)
    rr = np.repeat(r8.reshape(T, 8, 1, K), 16, axis=2).reshape(T, P, K)
    return np.ascontiguousarray(qw), np.ascontiguousarray(rr)


def kernel(values, node_constants, rationality, indices):
    global LAST_EXEC_NS
    trace = os.environ.get("KERNEL_TRACE", "") == "1"
    if trace:
        _install_ntff_hook()
    from concourse.bass_utils import run_bass_kernel_spmd

    values = np.asarray(values, dtype=np.float32)
    node_constants = np.asarray(node_constants, dtype=np.float32)
    indices = np.asarray(indices)
    r = float(np.asarray(rationality, dtype=np.float32))

    tab = np.ascontiguousarray(
        np.tile(node_constants.reshape(NE, 16).T, (8, 1))
    ).astype(np.float32)
    iota = np.tile(np.arange(16, dtype=np.float32), 8).reshape(P, 1)
    redw = np.zeros((P, 16), dtype=np.float32)
    for g in range(8):
        redw[16 * g : 16 * (g + 1), g] = 1.0
        redw[16 * g : 16 * (g + 1), 8 + g] = -1.0
    negr = np.full((P, 1), -r, dtype=np.float32)

    in_maps = []
    for c in range(NCORES):
        sl = slice(c * NNZ_CORE, (c + 1) * NNZ_CORE)
        idx = indices[sl]
        i32 = np.zeros(PAD, dtype=np.int32)
        j32 = np.zeros(PAD, dtype=np.int32)
        i32[:NNZ_CORE] = idx[:, 0].astype(np.int32)
        j32[:NNZ_CORE] = idx[:, 1].astype(np.int32)
        v = np.zeros(PAD, dtype=np.float32)
        v[:NNZ_CORE] = values[sl]
        qwi, rri = _prep_stream(i32)
        qwj, rrj = _prep_stream(j32)
        in_maps.append(
            {
                "tab": tab,
                "qwi": qwi,
                "qwj": qwj,
                "rri": rri,
                "rrj": rrj,
                "vals": np.ascontiguousarray(v.reshape(T, 8, K)),
                "iota": iota,
                "redw": redw,
                "negr": negr,
            }
        )

    nc = _build()
    res = run_bass_kernel_spmd(
        nc, in_maps, core_ids=list(range(NCORES)), trace=trace
    )
    LAST_EXEC_NS = res.exec_time_ns

    out = np.empty(NNZ, dtype=np.float32)
    for c in range(NCORES):
        out[c * NNZ_CORE : (c + 1) * NNZ_CORE] = res.results[c]["y"].reshape(PAD)[
            :NNZ_CORE
        ]
    return out

